# revision 6
# baseline (speedup 1.0000x reference)
"""MoE layer (N=8192, D=1024, F=4096, E=8, top-2) on 8 Trainium2 NeuronCores.

Strategy (expert-parallel + load balancing):
  - Host: gate (inputs @ Wg + bg), top-k selection, softmax combine weights,
    dispatch/combine index plumbing, and the w*b2 term.
  - Device (SPMD, core e ~ expert e): y = cw * (silu(x@W1+b1) @ W2) in bf16
    with fp32 PSUM accumulation.

Per-core capacity C = 2176 tokens = 2048 "own" tokens + one 128-token
overflow slot.  The overflow slot has its own streamed weights (w1x/w2x),
so a core whose expert has <2048 tokens can absorb another expert's
overflow; residual tokens that fit nowhere (a handful) are computed on
host in fp32.  This cuts the padded capacity from 2304 (max expert load
rounded up) to 2176 (balanced), i.e. ~5.6% less matmul streaming.

Device layout (all bf16 weights/activations, fp32 psum):
  blocks of [640, 640, 512, 384] tokens; overflow slot = last 128 of block 1.
  phase 1 per block: h^T[f,t] = silu(W1^T x^T + b1), W1 tile stationary.
  phase 2 per block: per token-tile K-contiguous: for tt: for dh: accumulate
    all 32 f-tiles into one PSUM bank, then scale by combine weight (VectorE)
    and DMA out.  This overlaps the output tail with subsequent matmuls.
  Startup: 22 dummy matmuls on a zeroed tile warm the PE (HAM) during the
    initial DMA wait; first real matmul needs only a 128-token x slice and
    one W1 f-tile.
  Queues: sync HWDGE = inputs (x, w1r, w1x); scalar HWDGE = w2r, w2x,
    consts, y out.
"""

import os
import sys
import types

import numpy as np

import concourse.bass as bass
import concourse.bacc as bacc
import concourse.mybir as mybir
import concourse.tile as tile
from concourse.bass_utils import run_bass_kernel_spmd


def _ensure_ntff_hook():
    """Provide antenv.axon_hooks if the image lacks it, so trace=True
    degrades gracefully instead of crashing in run_bass_kernel_spmd."""
    try:
        import antenv.axon_hooks  # noqa: F401

        return
    except ImportError:
        pass
    hook = None
    try:
        from trn_agent_boot.trn_boot import _ntff_profile_via_ctypes

        hook = _ntff_profile_via_ctypes("/opt/axon/libaxon_pjrt.so")
    except Exception:
        hook = None
    m = types.ModuleType("antenv.axon_hooks")
    m.get_axon_ntff_profile_hook = lambda: hook
    m.set_axon_ntff_profile_hook = lambda h: None
    sys.modules["antenv.axon_hooks"] = m
    try:
        import antenv

        antenv.axon_hooks = m
    except ImportError:
        pass


_ensure_ntff_hook()

F32 = mybir.dt.float32
BF16 = mybir.dt.bfloat16

D_MODEL = 1024
D_FF = 4096
N_EXPERTS = 8
N_CORES = 8

OWN_CAP = 2048  # own-expert token capacity per core
OVF_CAP = 128  # overflow slot (its own streamed weights)
C_TOK = OWN_CAP + OVF_CAP  # 2176

# blocks of tokens; block 1 ends with the 128-token overflow slot
BLOCKS = [640, 640, 512, 384]
# per-block phase-1 subtile widths (<=512 each); block1 last sub = overflow
SUBS = [[128, 512], [512, 128], [512], [384]]
OVF_BLOCK = 1  # overflow slot lives at the end of this block
N_WARMUP_MM = 22

LAST_EXEC_TIME_NS = None
_NC_CACHE = {}


def _build_nc():
    nc = bacc.Bacc("TRN2", target_bir_lowering=False, debug=False)
    D, F = D_MODEL, D_FF
    nf = F // 128  # 32
    nd = D // 128  # 8

    w1r = nc.declare_dram_parameter("w1r", [nf, 128, nd, 128], BF16, isOutput=False)
    w1x = nc.declare_dram_parameter("w1x", [nf, 128, nd, 128], BF16, isOutput=False)
    w2r = nc.declare_dram_parameter("w2r", [4, 128, nf // 4, D], BF16, isOutput=False)
    w2x = nc.declare_dram_parameter("w2x", [2, nf, 128, 512], BF16, isOutput=False)
    xds = [
        nc.declare_dram_parameter(f"x{i}", [128, nd, w], BF16, isOutput=False)
        for i, w in enumerate([128, 512, 640, 512, 384])
    ]
    b1r = nc.declare_dram_parameter("b1r", [128, nf], F32, isOutput=False)
    b1x = nc.declare_dram_parameter("b1x", [128, nf], F32, isOutput=False)
    cw = nc.declare_dram_parameter("cw", [128, C_TOK // 128], F32, isOutput=False)
    y = nc.declare_dram_parameter("y", [C_TOK, D], F32, isOutput=True)

    # which x dram param feeds each block (block0 split into two subs)
    blk_x = {0: [(xds[0], 0, 128), (xds[1], 128, 512)], 1: [(xds[2], 0, 640)],
             2: [(xds[3], 0, 512)], 3: [(xds[4], 0, 384)]}

    with tile.TileContext(nc) as tc:
        with (
            tc.tile_pool(name="const", bufs=1) as constp,
            tc.tile_pool(name="wres", bufs=1) as wres,
            tc.tile_pool(name="xp", bufs=2) as xp,
            tc.tile_pool(name="hp", bufs=1) as hp,
            tc.tile_pool(name="w1xp", bufs=2) as w1xp,
            tc.tile_pool(name="w2xp", bufs=3) as w2xp,
            tc.tile_pool(name="yp", bufs=2) as yp,
            tc.tile_pool(name="ps1", bufs=2, space="PSUM") as ps1,
            tc.tile_pool(name="ps2", bufs=6, space="PSUM") as ps2,
        ):
            # ---- PE warm-up: dummy matmuls on a zeroed tile so HAM sees
            # activity while the first input DMAs are in flight ----
            wz = constp.tile([128, 512], BF16, tag="wz")
            nc.vector.memset(wz[:], 0.0)
            pw = ps2.tile([128, 512], F32, tag="py")
            for _ in range(N_WARMUP_MM):
                nc.tensor.matmul(pw[:], wz[:, :128], wz[:], start=True, stop=True)

            # ---- resident weights ----
            w1r_sb = wres.tile([128, nf, nd, 128], BF16, tag="w1r")
            w2r_sb = wres.tile([128, 4, nf // 4, D], BF16, tag="w2r")
            # sync queue: first W1 f-tile, then x blocks 0/1 (xp pool gates
            # later blocks), then the rest of W1
            nc.sync.dma_start(w1r_sb[:, 0], w1r[0])

            x_tiles = []
            for bi, blk in enumerate(BLOCKS[:2]):
                x_sb = xp.tile([128, nd, 640], BF16, tag="x", name=f"x{bi}")
                for xd, s0, w in blk_x[bi]:
                    nc.sync.dma_start(x_sb[:, :, s0 : s0 + w], xd[:])
                x_tiles.append(x_sb)
            for f in range(1, nf):
                nc.sync.dma_start(w1r_sb[:, f], w1r[f])

            # scalar queue: biases/cw then resident W2
            b1r_sb = constp.tile([128, nf], F32, tag="b1r")
            nc.scalar.dma_start(b1r_sb[:], b1r[:])
            b1x_sb = constp.tile([128, nf], F32, tag="b1x")
            nc.scalar.dma_start(b1x_sb[:], b1x[:])
            cw_sb = constp.tile([128, C_TOK // 128], F32, tag="cw")
            nc.scalar.dma_start(cw_sb[:], cw[:])
            for c in range(4):
                nc.scalar.dma_start(w2r_sb[:, c], w2r[c])

            t0 = 0
            for bi, blk in enumerate(BLOCKS):
                ntt = blk // 128
                if bi < 2:
                    x_sb = x_tiles[bi]
                else:
                    x_sb = xp.tile([128, nd, 640], BF16, tag="x", name=f"x{bi}")
                    for xd, s0, w in blk_x[bi]:
                        nc.sync.dma_start(x_sb[:, :, s0 : s0 + w], xd[:])
                h_sb = hp.tile([128, nf, 640], BF16, tag="h")

                # ---- phase 1: h^T = silu(W1^T x^T + b1) ----
                subs = SUBS[bi]
                is_ovf_blk = bi == OVF_BLOCK

                def p1_group(f, s0, w, w1sel, b1sel):
                    ph = ps1.tile([128, 512], F32, tag="ph")
                    for d in range(nd):
                        nc.tensor.matmul(
                            ph[:, :w],
                            w1sel[:, d, :],
                            x_sb[:, d, s0 : s0 + w],
                            start=(d == 0),
                            stop=(d == nd - 1),
                        )
                    nc.scalar.activation(
                        h_sb[:, f, s0 : s0 + w],
                        ph[:, :w],
                        mybir.ActivationFunctionType.Silu,
                        bias=b1sel[:, f : f + 1],
                    )

                if not is_ovf_blk:
                    # sub-outer: first matmuls only need the first x slice
                    s0 = 0
                    for w in subs:
                        for f in range(nf):
                            p1_group(f, s0, w, w1r_sb[:, f], b1r_sb)
                        s0 += w
                else:
                    # f-outer so the 32 w1x tiles stream across the block
                    for f in range(nf):
                        w1x_t = w1xp.tile([128, nd, 128], BF16, tag="w1x")
                        nc.sync.dma_start(w1x_t[:], w1x[f])
                        s0 = 0
                        for si, w in enumerate(subs):
                            last = si == len(subs) - 1
                            p1_group(
                                f, s0, w,
                                w1x_t if last else w1r_sb[:, f],
                                b1x_sb if last else b1r_sb,
                            )
                            s0 += w

                # ---- phase 2: y = cw * ((h^T)^T @ W2) ----
                if not is_ovf_blk:
                    # K-contiguous per token tile: tail overlaps later matmuls
                    for tt in range(ntt):
                        for dh in range(2):
                            py = ps2.tile([128, 512], F32, tag="py")
                            for f in range(nf):
                                nc.tensor.matmul(
                                    py[:],
                                    h_sb[:, f, tt * 128 : (tt + 1) * 128],
                                    w2r_sb[:, f // 8, f % 8, dh * 512 : (dh + 1) * 512],
                                    start=(f == 0),
                                    stop=(f == nf - 1),
                                )
                            g = t0 // 128 + tt
                            y_sb = yp.tile([128, 512], F32, tag="y")
                            nc.vector.tensor_scalar_mul(
                                y_sb[:], py[:], cw_sb[:, g : g + 1]
                            )
                            nc.scalar.dma_start(
                                y[t0 + tt * 128 : t0 + (tt + 1) * 128,
                                  dh * 512 : (dh + 1) * 512],
                                y_sb[:],
                            )
                else:
                    # grouped over tts so the w2x stream spreads per f-tile;
                    # last tt uses the overflow weights
                    for dh in range(2):
                        pys = [
                            ps2.tile([128, 512], F32, tag="py", name=f"py{i}")
                            for i in range(ntt)
                        ]
                        for f in range(nf):
                            w2x_t = w2xp.tile([128, 512], BF16, tag="w2x")
                            nc.scalar.dma_start(w2x_t[:], w2x[dh, f])
                            for tt in range(ntt):
                                mov = (
                                    w2x_t[:]
                                    if tt == ntt - 1
                                    else w2r_sb[:, f // 8, f % 8,
                                                dh * 512 : (dh + 1) * 512]
                                )
                                nc.tensor.matmul(
                                    pys[tt][:],
                                    h_sb[:, f, tt * 128 : (tt + 1) * 128],
                                    mov,
                                    start=(f == 0),
                                    stop=(f == nf - 1),
                                )
                        for tt in range(ntt):
                            g = t0 // 128 + tt
                            y_sb = yp.tile([128, 512], F32, tag="y")
                            nc.vector.tensor_scalar_mul(
                                y_sb[:], pys[tt][:], cw_sb[:, g : g + 1]
                            )
                            nc.scalar.dma_start(
                                y[t0 + tt * 128 : t0 + (tt + 1) * 128,
                                  dh * 512 : (dh + 1) * 512],
                                y_sb[:],
                            )
                t0 += blk
    nc.finalize()
    return nc


def _route(inputs, Wg, bg, k):
    """Host gate: replicate reference numerics (fp32) for routing."""
    logits = inputs.astype(np.float32) @ Wg.astype(np.float32) + bg.astype(np.float32)
    sel = np.argsort(-logits, axis=1, kind="stable")[:, :k]  # == jax.lax.top_k order
    tl = np.take_along_axis(logits, sel, axis=1).astype(np.float32)
    m = tl.max(axis=1, keepdims=True)
    e = np.exp(tl - m, dtype=np.float32)
    w = (e / e.sum(axis=1, keepdims=True)).astype(np.float32)
    return sel, w


def _ffn_host(x, W1, b1, W2, b2):
    """fp32 FFN for the handful of tokens that fit no device slot."""
    h = x @ W1 + b1
    h = h * (1.0 / (1.0 + np.exp(-h)))
    return h @ W2 + b2


def kernel(inputs, Wg, bg, W1, b1, W2, b2, k):
    global LAST_EXEC_TIME_NS
    import ml_dtypes

    bf16 = ml_dtypes.bfloat16
    k = int(np.asarray(k))
    inputs = np.ascontiguousarray(np.asarray(inputs, dtype=np.float32))
    Wg = np.asarray(Wg, dtype=np.float32)
    bg = np.asarray(bg, dtype=np.float32)
    W1 = np.asarray(W1, dtype=np.float32)
    b1 = np.asarray(b1, dtype=np.float32)
    W2 = np.asarray(W2, dtype=np.float32)
    b2 = np.asarray(b2, dtype=np.float32)

    N, D = inputs.shape
    E = Wg.shape[1]
    assert E == N_EXPERTS and D == D_MODEL and W1.shape == (E, D, D_FF)

    sel, wts = _route(inputs, Wg, bg, k)

    # per-expert token lists
    idxs, wvals = [], []
    for e in range(E):
        tok, slot = np.nonzero(sel == e)
        idxs.append(tok)
        wvals.append(wts[tok, slot])

    # ---- placement: own tokens (up to OWN_CAP+OVF_CAP on own core), then
    # leftovers into other cores' free overflow slots, then host ----
    own = []  # per core: (orig_idx array, weight array) of own-expert tokens
    ovf = [None] * N_CORES  # per core: (expert, idx array, weight array)
    leftovers = []  # (expert, idx array, weight array)
    for e in range(E):
        ix, wv = idxs[e], wvals[e]
        own.append((ix[:OWN_CAP], wv[:OWN_CAP]))
        rem_i, rem_w = ix[OWN_CAP:], wv[OWN_CAP:]
        if len(rem_i):
            ovf[e] = (e, rem_i[:OVF_CAP], rem_w[:OVF_CAP])
            if len(rem_i) > OVF_CAP:
                leftovers.append((e, rem_i[OVF_CAP:], rem_w[OVF_CAP:]))
    host_list = []
    for e, ri, rw in leftovers:
        p = 0
        for c in range(N_CORES):
            if p >= len(ri):
                break
            if ovf[c] is None:
                take = min(OVF_CAP, len(ri) - p)
                ovf[c] = (e, ri[p : p + take], rw[p : p + take])
                p += take
        if p < len(ri):
            host_list.append((e, ri[p:], rw[p:]))

    # ---- per-core input maps ----
    in_maps = []
    books = []  # per core: (positions, orig idx, weights, expert-per-pos)
    sub_splits = [128, 640, 1280, 1792]  # x0|x1|x2|x3|x4 boundaries in [0,2176)
    for c in range(N_CORES):
        own_i, own_w = own[c]
        fe, ovf_i, ovf_w = ovf[c] if ovf[c] is not None else (c, own_i[:0], own_w[:0])
        xe = np.zeros((C_TOK, D), dtype=np.float32)
        cwe = np.zeros((C_TOK,), dtype=np.float32)
        # own tokens at positions [0:1152) and [1280:2176); overflow [1152:1280)
        own_pos = np.concatenate([np.arange(0, 1152), np.arange(1280, C_TOK)])
        pos_o = own_pos[: len(own_i)]
        xe[pos_o] = inputs[own_i]
        cwe[pos_o] = own_w
        pos_x = np.arange(1152, 1152 + len(ovf_i))
        xe[pos_x] = inputs[ovf_i]
        cwe[pos_x] = ovf_w
        books.append((pos_o, own_i, own_w, pos_x, ovf_i, ovf_w, fe))

        xeb = xe.astype(bf16)
        xparts = {}
        bounds = [0] + sub_splits + [C_TOK]
        for si in range(5):
            a, b = bounds[si], bounds[si + 1]
            xparts[f"x{si}"] = np.ascontiguousarray(
                xeb[a:b].reshape(b - a, 8, 128).transpose(2, 1, 0)
            )
        w1r_h = np.ascontiguousarray(
            W1[c].astype(bf16).reshape(8, 128, 32, 128).transpose(2, 1, 0, 3)
        )
        w1x_h = np.ascontiguousarray(
            W1[fe].astype(bf16).reshape(8, 128, 32, 128).transpose(2, 1, 0, 3)
        )
        w2r_h = np.ascontiguousarray(
            W2[c].astype(bf16).reshape(4, 8, 128, D).transpose(0, 2, 1, 3)
        )
        w2x_h = np.ascontiguousarray(
            W2[fe].astype(bf16).reshape(32, 128, 2, 512).transpose(2, 0, 1, 3)
        )
        b1r_h = np.ascontiguousarray(b1[c].reshape(32, 128).T)
        b1x_h = np.ascontiguousarray(b1[fe].reshape(32, 128).T)
        cw_h = np.ascontiguousarray(cwe.reshape(C_TOK // 128, 128).T)
        m = {"w1r": w1r_h, "w1x": w1x_h, "w2r": w2r_h, "w2x": w2x_h,
             "b1r": b1r_h, "b1x": b1x_h, "cw": cw_h}
        m.update(xparts)
        in_maps.append(m)

    if "nc" not in _NC_CACHE:
        _NC_CACHE["nc"] = _build_nc()
    nc = _NC_CACHE["nc"]

    trace = bool(os.environ.get("BASS_TRACE"))
    res = None
    for attempt in range(3):
        try:
            res = run_bass_kernel_spmd(
                nc, in_maps, core_ids=list(range(N_CORES)), trace=trace
            )
            break
        except Exception:
            if attempt == 2:
                raise
            import time

            time.sleep(20)
    LAST_EXEC_TIME_NS = getattr(res, "exec_time_ns", None)

    results = np.zeros((N, D), dtype=np.float32)
    for c in range(N_CORES):
        pos_o, own_i, own_w, pos_x, ovf_i, ovf_w, fe = books[c]
        ye = np.asarray(res.results[c]["y"])
        # device computed cw * (silu(x W1 + b1) @ W2); add cw * b2 here
        np.add.at(results, own_i, ye[pos_o] + own_w[:, None] * b2[c][None, :])
        if len(ovf_i):
            np.add.at(results, ovf_i, ye[pos_x] + ovf_w[:, None] * b2[fe][None, :])
    for e, ri, rw in host_list:
        ye = _ffn_host(inputs[ri], W1[e], b1[e], W2[e], b2[e])
        np.add.at(results, ri, rw[:, None] * ye)
    return results.astype(np.float32)
